# revision 3
# baseline (speedup 1.0000x reference)
"""Multi-head attention (S=2048, B=2, D=1024, H=16, DH=64) on 8 Trainium2 cores.

Sharding: head-parallel tensor parallelism. Core c owns heads {2c, 2c+1}
(feature slice [128c, 128c+128) of the QKV projections / Wo input rows).
Each core computes QKV for its heads over all tokens, full attention for its
4 (batch, head) pairs, then an AllToAll reshards by token so each core runs
1/8 of the output projection on its own token slice.

Layouts (tokens ordered (b, s), i.e. token = b*S + s):
  xT       [D, NTOK]            (host pre-transposed)
  qT/kT    [128 feat, NTOK]     head A on partitions 0:64, head B on 64:128
  scoresT  [t, s] tiles         via row-paired K=64 matmuls (both heads run
                                concurrently in disjoint PE row groups)
  V_aug    [t, 65] per head     65th column of ones => softmax denominator
                                accumulates in PSUM row 64 for free
  attn out [feat, tok] -> A2A -> out rows [tok, D]
"""

import os
import time

import numpy as np
import ml_dtypes

BF16 = ml_dtypes.bfloat16

S, B, D = 2048, 2, 1024
H, DH = 16, 64
N_CORES = 8
FPC = (H // N_CORES) * DH  # 128 features per core (2 heads)
SCALE = DH ** -0.5


def build_program(s=S, b_sz=B, debug=False, reps=1, no_collective=False):
    """Build the per-core Bass/Tile program (same program on all 8 cores)."""
    import concourse.bass as bass
    import concourse.mybir as mybir
    import concourse.tile as tile
    from concourse import bacc
    from concourse.masks import make_identity

    f32 = mybir.dt.float32
    f32r = mybir.dt.float32r
    bf16 = mybir.dt.bfloat16
    AF = mybir.ActivationFunctionType
    OP = mybir.AluOpType

    ntok = s * b_sz
    JC = ntok // 512      # token chunks for QKV projections
    KT = D // 128         # contraction tiles over D
    MT = ntok // 128      # token tiles for V transpose
    SC = s // 512         # s-chunks per batch (attention column blocks)
    TT = s // 128         # t-tiles per batch
    TPC = s // N_CORES    # output tokens per core per batch
    OTT = (TPC + 127) // 128  # output token tiles per batch

    nc = bacc.Bacc("TRN2", target_bir_lowering=False, debug=False,
                   num_devices=N_CORES)

    # ---- kernel I/O -------------------------------------------------------
    xT_e = nc.dram_tensor("xT", [D, ntok], f32r, kind="ExternalInput")
    wqT_e = nc.dram_tensor("wqT", [D, FPC], f32r, kind="ExternalInput")
    wkT_e = nc.dram_tensor("wkT", [D, FPC], f32r, kind="ExternalInput")
    wvT_e = nc.dram_tensor("wvT", [D, FPC], f32r, kind="ExternalInput")
    woT_e = nc.dram_tensor("woT", [D, D], bf16, kind="ExternalInput")
    bq_e = nc.dram_tensor("bq", [FPC, 1], f32, kind="ExternalInput")
    bk_e = nc.dram_tensor("bk", [FPC, 1], f32, kind="ExternalInput")
    bv_e = nc.dram_tensor("bv", [FPC, 1], f32, kind="ExternalInput")
    bo_e = nc.dram_tensor("bo", [1, D], bf16, kind="ExternalInput")
    out_e = nc.dram_tensor("out", [b_sz * TPC, D], f32, kind="ExternalOutput")

    rg = [list(range(N_CORES))]

    def single(shape, dtype, name):
        return persist.tile(shape, dtype, name=name, tag=name)

    with tile.TileContext(nc) as tc:
        from contextlib import ExitStack
        with ExitStack() as ctx:
            persist = ctx.enter_context(
                tc.tile_pool(name="persist", bufs=1))
            dram = ctx.enter_context(
                tc.tile_pool(name="dram", bufs=1, space="DRAM"))
            x_pool = ctx.enter_context(tc.tile_pool(name="x_pool", bufs=16))
            e_pool = ctx.enter_context(tc.tile_pool(name="e_pool", bufs=6))
            l_pool = ctx.enter_context(tc.tile_pool(name="l_pool", bufs=16))
            of_pool = ctx.enter_context(tc.tile_pool(name="of_pool", bufs=4))
            rep_pool = ctx.enter_context(tc.tile_pool(name="rep_pool", bufs=2))
            ps_qkv = ctx.enter_context(
                tc.tile_pool(name="ps_qkv", bufs=1, space="PSUM"))
            ps_vt = ctx.enter_context(
                tc.tile_pool(name="ps_vt", bufs=1, space="PSUM"))
            ps_s = ctx.enter_context(
                tc.tile_pool(name="ps_s", bufs=4, space="PSUM"))
            ps_o = ctx.enter_context(
                tc.tile_pool(name="ps_o", bufs=2, space="PSUM"))

            # ---- persistent SBUF tensors ---------------------------------
            wq_t = [single([128, FPC], f32r, f"wq{k}") for k in range(KT)]
            wk_t = [single([128, FPC], f32r, f"wk{k}") for k in range(KT)]
            wv_t = [single([128, FPC], f32r, f"wv{k}") for k in range(KT)]
            wo_t = [single([128, D], bf16, f"wo{k}") for k in range(KT)]
            bq_t = single([FPC, 1], f32, "bq_t")
            bk_t = single([FPC, 1], f32, "bk_t")
            bv_t = single([FPC, 1], f32, "bv_t")
            bo_t = single([1, D], bf16, "bo_t")
            ones128b = single([1, 128], bf16, "ones128b")
            ident_b = single([128, 128], bf16, "ident_b")
            qT = single([FPC, ntok], f32r, "qT")
            kT = single([FPC, ntok], f32r, "kT")
            vT = single([FPC, ntok], bf16, "vT")
            V_sb = single([128, 130 * MT], bf16, "V_sb")
            attn_un = single([128, ntok], bf16, "attn_un")
            attn_nm = single([128, ntok], bf16, "attn_nm")
            # softmax denominators: row 32*(2b+h) (32-aligned start
            # partitions), column block sc*512. Rows memset to 1.0 so the
            # in-place exp(-ln(x)) reciprocal stays finite on unused rows.
            den_bs = single([128, SC * 512], f32, "den_bs")

            den_d = dram.tile([2 * b_sz, SC * 512], f32, name="den_d")
            a2a_in = [dram.tile([N_CORES, 128, TPC], bf16, name=f"a2ai{b}")
                      for b in range(b_sz)]
            a2a_out = [dram.tile([N_CORES, 128, TPC], bf16, name=f"a2ao{b}")
                       for b in range(b_sz)]

            # ---- constants + weight loads --------------------------------
            nc.gpsimd.memset(ones128b[:], 1.0)
            nc.gpsimd.memset(V_sb[:], 1.0)  # pre-set: ones cols 64/129 survive
            nc.gpsimd.memset(den_bs[:], 1.0)
            make_identity(nc, ident_b[:])
            for k in range(KT):
                nc.sync.dma_start(wq_t[k][:], wqT_e[128 * k:128 * (k + 1), :])
                nc.sync.dma_start(wk_t[k][:], wkT_e[128 * k:128 * (k + 1), :])
                nc.sync.dma_start(wv_t[k][:], wvT_e[128 * k:128 * (k + 1), :])
                nc.sync.dma_start(wo_t[k][:], woT_e[128 * k:128 * (k + 1), :])
            nc.sync.dma_start(bq_t[:], bq_e[:])
            nc.sync.dma_start(bk_t[:], bk_e[:])
            nc.sync.dma_start(bv_t[:], bv_e[:])
            nc.sync.dma_start(bo_t[:], bo_e[:])

            for _rep in range(reps):
                # ---- phase 1: QKV projections (+ V transpose) ----------------
                for j in range(JC):
                    js = slice(512 * j, 512 * (j + 1))
                    x_t = []
                    for k in range(KT):
                        xt = x_pool.tile([128, 512], f32r, name="xt", tag="xt")
                        nc.sync.dma_start(xt[:], xT_e[128 * k:128 * (k + 1), js])
                        x_t.append(xt)
                    for w_t, b_t, dstT in ((wq_t, bq_t, qT), (wk_t, bk_t, kT),
                                           (wv_t, bv_t, vT)):
                        ps = ps_qkv.tile([128, 512], f32, name="psq", tag="psq")
                        for k in range(KT):
                            nc.tensor.matmul(ps[:], w_t[k][:], x_t[k][:],
                                             start=(k == 0), stop=(k == KT - 1))
                        nc.vector.tensor_scalar_add(dstT[:, js], ps[:], b_t[:])
                    # V transpose for the token tiles this chunk completed
                    for m in range(4 * j, 4 * (j + 1)):
                        pst = ps_vt.tile([128, 1024], bf16, name="pst", tag="pst")
                        nc.tensor.transpose(pst[:, 0:128],
                                            vT[:, 128 * m:128 * (m + 1)],
                                            ident_b[:])
                        c0 = 130 * m
                        nc.vector.tensor_copy(V_sb[:, c0:c0 + 64],
                                              pst[:, 0:64])
                        nc.vector.tensor_copy(V_sb[:, c0 + 65:c0 + 129],
                                              pst[:, 64:128])

                # ---- phase 2: attention per (batch, s-chunk) -----------------
                for b in range(b_sz):
                    for sc in range(SC):
                        s0 = b * s + 512 * sc
                        ss = slice(s0, s0 + 512)
                        psO_A = ps_o.tile([128, 512], f32, name="psoa", tag="pso")
                        psO_B = ps_o.tile([128, 512], f32, name="psob", tag="pso")
                        for t in range(TT):
                            t0 = b * s + 128 * t
                            ts_ = slice(t0, t0 + 128)
                            psA = ps_s.tile([128, 512], f32, name="psa", tag="pss")
                            psB = ps_s.tile([128, 512], f32, name="psb", tag="pss")
                            nc.tensor.matmul(psA[:], kT[0:64, ts_],
                                             qT[0:64, ss],
                                             start=True, stop=True,
                                             tile_position=(0, 0))
                            nc.tensor.matmul(psB[:], kT[64:128, ts_],
                                             qT[64:128, ss],
                                             start=True, stop=True,
                                             tile_position=(64, 0))
                            eA = e_pool.tile([128, 512], bf16, name="ea", tag="e")
                            eB = e_pool.tile([128, 512], bf16, name="eb", tag="e")
                            nc.scalar.activation(eA[:], psA[:], AF.Exp, scale=SCALE)
                            nc.scalar.activation(eB[:], psB[:], AF.Exp, scale=SCALE)
                            m = (b * s) // 128 + t
                            nc.tensor.matmul(psO_A[0:65, :],
                                             V_sb[:, 130 * m:130 * m + 65],
                                             eA[:],
                                             start=(t == 0), stop=(t == TT - 1))
                            nc.tensor.matmul(psO_B[0:65, :],
                                             V_sb[:, 130 * m + 65:130 * m + 130],
                                             eB[:],
                                             start=(t == 0), stop=(t == TT - 1))
                        ra, rb = 64 * b, 64 * b + 32
                        blk = slice(sc * 512, (sc + 1) * 512)
                        nc.vector.tensor_copy(den_bs[ra:ra + 1, blk],
                                              psO_A[64:65, :])
                        nc.vector.tensor_copy(den_bs[rb:rb + 1, blk],
                                              psO_B[64:65, :])
                        nc.vector.tensor_copy(attn_un[0:64, ss], psO_A[0:64, :])
                        nc.vector.tensor_copy(attn_un[64:128, ss], psO_B[0:64, :])

                    # reciprocal in place: x -> exp(-ln(x)) on ACT (full-rate;
                    # DVE's iterative-divide recip measures ~12.8 ns/elem/lane)
                    nc.scalar.activation(den_bs[:], den_bs[:], AF.Ln)
                    nc.scalar.activation(den_bs[:], den_bs[:], AF.Exp, scale=-1.0)
                    for h in range(2):
                        r = 64 * b + 32 * h
                        nc.sync.dma_start(den_d[2 * b + h, :],
                                          den_bs[r:r + 1, :].opt())
                    for sc in range(SC):
                        s0 = b * s + 512 * sc
                        ss = slice(s0, s0 + 512)
                        ra, rb = 64 * b, 64 * b + 32
                        blk = slice(sc * 512, (sc + 1) * 512)
                        rep = rep_pool.tile([128, 512], f32, name="rep", tag="rep")
                        nc.sync.dma_start(
                            rep[0:64, :],
                            den_d[2 * b:2 * b + 1, blk].broadcast_to([64, 512]))
                        nc.sync.dma_start(
                            rep[64:128, :],
                            den_d[2 * b + 1:2 * b + 2, blk].broadcast_to(
                                [64, 512]))
                        nc.vector.scalar_tensor_tensor(
                            attn_nm[:, ss], attn_un[:, ss], 1.0, rep[:],
                            op0=OP.bypass, op1=OP.mult)

                    # A2A reshard: (feat-shard, all tokens) -> (all feat, my toks)
                    bs = slice(b * s, (b + 1) * s)
                    nc.sync.dma_start(
                        a2a_in[b][:].rearrange("c p t -> p c t"),
                        attn_nm[:, bs].rearrange("p (c t) -> p c t", c=N_CORES))
                    if no_collective:
                        nc.sync.dma_start(a2a_out[b][:], a2a_in[b][:])
                    else:
                        nc.gpsimd.collective_compute(
                            "AllToAll", OP.bypass, replica_groups=rg,
                            ins=[a2a_in[b].opt()], outs=[a2a_out[b].opt()])

                    # output projection for my token slice of this batch
                    l_t = []
                    for kv in range(KT):
                        lt = l_pool.tile([128, TPC], bf16, name="lt", tag="lt")
                        nc.sync.dma_start(lt[:], a2a_out[b][kv])
                        l_t.append(lt)
                    for tt in range(OTT):
                        p = min(128, TPC - 128 * tt)
                        for dc in range(2):
                            ds_ = slice(512 * dc, 512 * (dc + 1))
                            psF = ps_s.tile([128, 512], f32, name="psf", tag="pss")
                            for kv in range(KT):
                                nc.tensor.matmul(
                                    psF[0:p, :],
                                    l_t[kv][:, 128 * tt:128 * tt + p],
                                    wo_t[kv][:, ds_],
                                    start=(kv == 0), stop=False)
                            nc.tensor.matmul(psF[0:p, :], ones128b[0:1, 0:p],
                                             bo_t[0:1, ds_],
                                             start=False, stop=True)
                            of = of_pool.tile([128, 512], f32, name="of", tag="of")
                            nc.vector.tensor_copy(of[0:p, :], psF[0:p, :])
                            r0 = b * TPC + 128 * tt
                            nc.sync.dma_start(out_e[r0:r0 + p, ds_], of[0:p, :])

            if debug:
                for nm, t_, shp, dt_ in (
                        ("qT", qT, [FPC, ntok], f32r),
                        ("den", den_bs, [128, SC * 512], f32),
                        ("vsb", V_sb, [128, 130 * MT], bf16),
                        ("aun", attn_un, [128, ntok], bf16),
                        ("anm", attn_nm, [128, ntok], bf16)):
                    d_e = nc.dram_tensor(f"dbg_{nm}", shp, dt_,
                                         kind="ExternalOutput")
                    nc.sync.dma_start(d_e[:], t_[:])

    nc.compile()
    return nc


# --------------------------------------------------------------------------
# host side: sharding, execution, unsharding
# --------------------------------------------------------------------------
_CACHE = {}


def _get_runner(s=S, b_sz=B, debug=False, reps=1):
    """Compile once; return a callable that executes the SPMD program on the
    8 axon-attached NeuronCores and returns per-core output dicts."""
    key = (s, b_sz, debug, reps)
    if key in _CACHE:
        return _CACHE[key]

    import jax
    import jax.numpy as jnp
    from jax.sharding import Mesh, PartitionSpec
    from jax.experimental.shard_map import shard_map
    import concourse.mybir as mybir
    from concourse import bass2jax

    nc = build_program(s, b_sz, debug=debug, reps=reps)
    bass2jax.install_neuronx_cc_hook()

    part_name = nc.partition_id_tensor.name if nc.partition_id_tensor else None
    in_names, out_names, out_avals = [], [], []
    for alloc in nc.m.functions[0].allocations:
        if not isinstance(alloc, mybir.MemoryLocationSet):
            continue
        name = alloc.memorylocations[0].name
        if alloc.kind == "ExternalInput":
            if name != part_name:
                in_names.append(name)
        elif alloc.kind == "ExternalOutput":
            out_names.append(name)
            out_avals.append(jax.core.ShapedArray(
                tuple(alloc.tensor_shape), mybir.dt.np(alloc.dtype)))
    n_params = len(in_names)
    all_names = list(in_names) + list(out_names)
    if part_name is not None:
        all_names.append(part_name)

    def _body(*args):
        operands = list(args)
        if part_name is not None:
            operands.append(bass2jax.partition_id_tensor())
        outs = bass2jax._bass_exec_p.bind(
            *operands, out_avals=tuple(out_avals), in_names=tuple(all_names),
            out_names=tuple(out_names), lowering_input_output_aliases=(),
            sim_require_finite=True, sim_require_nnan=True, nc=nc)
        return tuple(outs)

    devices = jax.devices()[:N_CORES]
    mesh = Mesh(np.asarray(devices), ("core",))
    n_outs = len(out_names)
    fn = jax.jit(
        shard_map(_body, mesh=mesh,
                  in_specs=(PartitionSpec("core"),) * (n_params + n_outs),
                  out_specs=(PartitionSpec("core"),) * n_outs,
                  check_rep=False),
        donate_argnums=tuple(range(n_params, n_params + n_outs)),
        keep_unused=True)

    def runner(in_maps, iters=1):
        concat = [np.concatenate([np.asarray(m[nm]) for m in in_maps], axis=0)
                  for nm in in_names]
        args = [jax.device_put(a) for a in concat]
        for a in args:
            a.block_until_ready()

        def zeros():
            return [jnp.zeros((N_CORES * av.shape[0], *av.shape[1:]),
                              av.dtype) for av in out_avals]

        t0 = time.perf_counter()
        outs = fn(*args, *zeros())
        jax.block_until_ready(outs)
        t_first = time.perf_counter() - t0

        t_iter = t_first
        if iters > 1:
            zs = [zeros() for _ in range(iters)]
            jax.block_until_ready(zs)
            t0 = time.perf_counter()
            for i in range(iters):
                outs = fn(*args, *zs[i])
            jax.block_until_ready(outs)
            t_iter = (time.perf_counter() - t0) / iters

        res = [{nm: np.asarray(outs[i]).reshape(N_CORES, *out_avals[i].shape)[c]
                for i, nm in enumerate(out_names)} for c in range(N_CORES)]
        return res, t_first, t_iter

    _CACHE[key] = runner
    return runner


def make_in_maps(x, Wq, bq, Wk, bk, Wv, bv, Wo, bo, s=S, b_sz=B):
    """Full inputs -> per-core input dicts (the sharding step)."""
    x = np.asarray(x, np.float32)
    ntok = s * b_sz
    # token order (b, s)
    x_bs = np.ascontiguousarray(x.transpose(1, 0, 2).reshape(ntok, D))
    xT = np.ascontiguousarray(x_bs.T)                      # [D, NTOK]
    woT = np.ascontiguousarray(np.asarray(Wo, np.float32).T).astype(BF16)
    bo_r = np.asarray(bo, np.float32).reshape(1, D).astype(BF16)
    in_maps = []
    for c in range(N_CORES):
        fs = slice(FPC * c, FPC * (c + 1))
        in_maps.append({
            "xT": xT,
            "wqT": np.ascontiguousarray(np.asarray(Wq, np.float32)[fs, :].T),
            "wkT": np.ascontiguousarray(np.asarray(Wk, np.float32)[fs, :].T),
            "wvT": np.ascontiguousarray(np.asarray(Wv, np.float32)[fs, :].T),
            "woT": woT,
            "bq": np.asarray(bq, np.float32)[fs].reshape(FPC, 1),
            "bk": np.asarray(bk, np.float32)[fs].reshape(FPC, 1),
            "bv": np.asarray(bv, np.float32)[fs].reshape(FPC, 1),
            "bo": bo_r,
        })
    return in_maps


def assemble_output(res, s=S, b_sz=B):
    """Per-core [B*TPC, D] row blocks -> full [S, B, D] output."""
    tpc = s // N_CORES
    out_bs = np.empty((b_sz, s, D), np.float32)
    for c in range(N_CORES):
        rc = res[c]["out"].reshape(b_sz, tpc, D)
        out_bs[:, tpc * c:tpc * (c + 1), :] = rc
    return np.ascontiguousarray(out_bs.transpose(1, 0, 2))


def kernel(x, Wq, bq, Wk, bk, Wv, bv, Wo, bo):
    assert x.shape == (S, B, D), x.shape
    runner = _get_runner()
    in_maps = make_in_maps(x, Wq, bq, Wk, bk, Wv, bv, Wo, bo)
    res, _, _ = runner(in_maps)
    return assemble_output(res)


def kernel_timed(x, Wq, bq, Wk, bk, Wv, bv, Wo, bo, iters=8):
    runner = _get_runner()
    in_maps = make_in_maps(x, Wq, bq, Wk, bk, Wv, bv, Wo, bo)
    res, t_first, t_iter = runner(in_maps, iters=iters)
    return assemble_output(res), t_first, t_iter

